# revision 1
# baseline (speedup 1.0000x reference)
"""Trainium2 Bass kernel for nn_CustomMSELoss (penalty-weighted MSE - variance).

loss = mean(penalty * (y_true - y_pred)^2) - var(y_pred, ddof=1)
  penalty = 6 where y_true < percentile(y_true, 15)
          = 6 where y_true > percentile(y_true, 85)
          = 1 otherwise

Strategy (8 NeuronCores, data-parallel over the element axis):
  Each core streams its 1/8 shard of (y_true, y_pred) once from HBM and
  computes, fully fused per 128x2048 tile:
    - sum(r^2)                    (ACT Square + hardware accumulator)
    - sum(y_pred^2)               (ACT Square + accumulator)
    - |y_true|                    (ACT Abs; feeds the mid-band mask)
    - sum(r^2 * [|y_true|<=T_MID])  (DVE scalar_tensor_tensor + accumulator)
    - #(y_true < -T_OUT), #(y_true > +T_OUT)   (DVE tensor_scalar + accumulator)
    - sum(y_pred)                 (PE ones-matmul accumulating in PSUM)
  Since LEFT_PENALTY == RIGHT_PENALTY, only the combined tail sum
  sum(r^2) - sum(r^2 * mid) is needed, with an exact host-side correction for
  elements near the percentile boundaries.

  The exact percentiles are order statistics. The device supplies exact
  global counts below/above +-T_OUT; the host ranks the order statistic
  inside the narrow value band (T_IN..T_OUT, ~1% of elements) and applies
  the exact r^2 correction for elements between the fixed device threshold
  T_MID and the true percentile thresholds. All arithmetic that must match
  the device (subtract, square, abs, compares) is replayed in float32.
  If the band does not contain the percentile ranks (pathological input
  distribution), falls back to an exact host computation.
"""

import os
import sys

import numpy as np

# ---------------------------------------------------------------- constants
N_TOTAL = 33554432
NCORES = 8
SHARD = N_TOTAL // NCORES          # 4_194_304
P = 128                            # SBUF partitions
F = 2048                           # tile free dim
NTILES = SHARD // (P * F)          # 16
MM_N = 512                         # matmul free-dim chunk

LEFT_PCT = 15.0
RIGHT_PCT = 85.0
PENALTY = 6.0
VAR_W = 1.0

# Fixed value-band thresholds around the expected +-1.0364 percentiles of
# N(0,1).  T_MID is the on-device penalty-mask boundary; the host corrects
# exactly within the (T_IN, T_OUT) band, which must contain T_MID and both
# true percentile values.
T_MID = np.float32(1.04)
T_IN = np.float32(1.025)
T_OUT = np.float32(1.055)

_CONCOURSE_PATHS = ["/opt/trn_rl_repo", "/root/.axon_site/_ro/trn_rl_repo"]


def _import_concourse():
    try:
        import concourse.bass  # noqa: F401
    except ImportError:
        for p in _CONCOURSE_PATHS:
            if os.path.isdir(p) and p not in sys.path:
                sys.path.insert(0, p)
        import concourse.bass  # noqa: F401


# ---------------------------------------------------------------- device IR
_NC_CACHE = {}

# engine assignment / buffering chosen from HW A/B timing
BEST_CFG = dict(sub_engine="vector", dma_engine="gpsimd", io_bufs=4, mid_bufs=3)


def build_nc(shard=SHARD, f=F, ntiles=None, repeat=1, sub_engine="vector",
             dma_engine="gpsimd", io_bufs=3, mid_bufs=2, dma_span=1,
             en_dma=True, en_dve=True, en_act=True, en_pe=True):
    """Build the per-core Bass program (identical on all cores).

    repeat>1 re-runs the whole streaming pass (for HW timing via wall-clock
    deltas); outputs stay valid since accumulator slots are overwritten.
    """
    _import_concourse()
    from contextlib import ExitStack

    import concourse.bacc as bacc
    import concourse.bass as bass  # noqa: F401
    import concourse.tile as tile
    from concourse import mybir

    if ntiles is None:
        ntiles = shard // (P * f)
    assert shard == P * f * ntiles

    assert ntiles % dma_span == 0
    key = (shard, f, ntiles, repeat, sub_engine, dma_engine, io_bufs, mid_bufs,
           dma_span, en_dma, en_dve, en_act, en_pe)
    if key in _NC_CACHE:
        return _NC_CACHE[key]

    fp32 = mybir.dt.float32
    Alu = mybir.AluOpType
    Act = mybir.ActivationFunctionType

    nc = bacc.Bacc()
    yt_d = nc.declare_dram_parameter("y_true", [shard], fp32, isOutput=False)
    yp_d = nc.declare_dram_parameter("y_pred", [shard], fp32, isOutput=False)
    out_acc = nc.declare_dram_parameter("acc", [P, 5 * ntiles], fp32, isOutput=True)
    out_yps = nc.declare_dram_parameter("ypsum", [1, MM_N], fp32, isOutput=True)

    # DMA granularity: dma_span compute-tiles per dma_start
    fd = f * dma_span
    ytv = yt_d[:].rearrange("(n p f) -> n p f", p=P, f=fd)
    ypv = yp_d[:].rearrange("(n p f) -> n p f", p=P, f=fd)

    with ExitStack() as ctx:
        tc = ctx.enter_context(tile.TileContext(nc))
        if repeat > 1:
            # timing builds: loop the whole streaming pass on-device so the
            # kernel's execution dominates wall-clock
            ctx.enter_context(tc.For_i(0, repeat, 1))
        io = ctx.enter_context(tc.tile_pool(name="io", bufs=io_bufs))
        mid = ctx.enter_context(tc.tile_pool(name="mid", bufs=mid_bufs))
        scr = ctx.enter_context(tc.tile_pool(name="scr", bufs=1))
        accp = ctx.enter_context(tc.tile_pool(name="accp", bufs=1))
        psp = ctx.enter_context(tc.tile_pool(name="psum", bufs=1, space="PSUM"))

        # acc layout along free dim: [r2 | yp2 | smid | cl | cr] x ntiles
        acc = accp.tile([P, 5 * ntiles], fp32)
        scr_dve = scr.tile([P, f], fp32)
        scr_act = scr.tile([P, f], fp32)
        ones = accp.tile([P, 1], fp32)
        nc.vector.memset(ones, 1.0)
        ps = psp.tile([1, MM_N], fp32)

        a_r2 = lambda t: acc[:, 0 * ntiles + t : 0 * ntiles + t + 1]
        a_yp2 = lambda t: acc[:, 1 * ntiles + t : 1 * ntiles + t + 1]
        a_smid = lambda t: acc[:, 2 * ntiles + t : 2 * ntiles + t + 1]
        a_cl = lambda t: acc[:, 3 * ntiles + t : 3 * ntiles + t + 1]
        a_cr = lambda t: acc[:, 4 * ntiles + t : 4 * ntiles + t + 1]

        for rep in range(1):
          for td in range(ntiles // dma_span):
            ytd = io.tile([P, fd], fp32, tag="yt")
            ypd = io.tile([P, fd], fp32, tag="yp")
            if en_dma:
                getattr(nc, dma_engine).dma_start(out=ytd, in_=ytv[td])
                getattr(nc, dma_engine).dma_start(out=ypd, in_=ypv[td])
            for ts in range(dma_span):
              t = td * dma_span + ts
              yt = ytd[:, ts * f : (ts + 1) * f]
              yp = ypd[:, ts * f : (ts + 1) * f]

              r = mid.tile([P, f], fp32, tag="r")
              if en_dve:
                  getattr(nc, sub_engine).tensor_sub(r, yt, yp)

              ayt = mid.tile([P, f], fp32, tag="ayt")
              r2 = mid.tile([P, f], fp32, tag="r2")
              if en_act:
                  nc.scalar.activation(ayt, yt, Act.Abs)
                  nc.scalar.activation(r2, r, Act.Square, accum_out=a_r2(t))
                  nc.scalar.activation(scr_act, yp, Act.Square,
                                       accum_out=a_yp2(t))

              if en_dve:
                  # (|y_t| <= T_MID) * r^2  summed per partition
                  nc.vector.scalar_tensor_tensor(
                      scr_dve, ayt, float(T_MID), r2, Alu.is_le, Alu.mult,
                      accum_out=a_smid(t),
                  )
                  # counts outside the +-T_OUT band
                  nc.vector.tensor_scalar(
                      scr_dve, yt, -float(T_OUT), None, Alu.is_lt, Alu.add,
                      accum_out=a_cl(t),
                  )
                  nc.vector.tensor_scalar(
                      scr_dve, yt, float(T_OUT), None, Alu.is_gt, Alu.add,
                      accum_out=a_cr(t),
                  )

              # sum(y_pred) on the otherwise-idle PE: ones^T @ yp chunks,
              # accumulated in a single PSUM region across all tiles
              n_mm = f // MM_N
              if en_pe:
                for c in range(n_mm):
                  nc.tensor.matmul(
                      ps[:, :],
                      ones,
                      yp[:, c * MM_N : (c + 1) * MM_N],
                      start=(t == 0 and c == 0),
                      stop=(t == ntiles - 1 and c == n_mm - 1),
                  )

        # write back results
        yps_sb = accp.tile([1, MM_N], fp32)
        if en_pe:
            nc.vector.tensor_copy(yps_sb, ps)
        nc.gpsimd.dma_start(out=out_acc[:, :], in_=acc)
        nc.gpsimd.dma_start(out=out_yps[:, :], in_=yps_sb)

    nc.finalize()
    _NC_CACHE[key] = nc
    return nc


# ---------------------------------------------------------------- device run
def run_device(y_pred, y_true, trace=False):
    """Shard across 8 cores, run the Bass kernel, return per-core outputs."""
    _import_concourse()
    from concourse.bass_utils import run_bass_kernel_spmd

    nc = build_nc(**BEST_CFG)
    in_maps = []
    for i in range(NCORES):
        sl = slice(i * SHARD, (i + 1) * SHARD)
        in_maps.append(
            {
                "y_true": np.ascontiguousarray(y_true[sl]),
                "y_pred": np.ascontiguousarray(y_pred[sl]),
            }
        )
    res = run_bass_kernel_spmd(nc, in_maps, list(range(NCORES)), trace=trace)
    return res


def _combine(results):
    """Combine per-core device partials (float64)."""
    acc = np.stack([np.asarray(r["acc"], dtype=np.float64) for r in results])
    nt = acc.shape[-1] // 5
    s_r2 = acc[:, :, 0 * nt : 1 * nt].sum()
    s_yp2 = acc[:, :, 1 * nt : 2 * nt].sum()
    s_mid = acc[:, :, 2 * nt : 3 * nt].sum()
    c_l = acc[:, :, 3 * nt : 4 * nt].sum()
    c_r = acc[:, :, 4 * nt : 5 * nt].sum()
    s_yp = np.stack([np.asarray(r["ypsum"], dtype=np.float64) for r in results]).sum()
    return s_r2, s_yp2, s_mid, c_l, c_r, s_yp


# ------------------------------------------------------------- host finishing
def _f32_percentile_pos(n, pct):
    """Replicate jnp.percentile's float32 position arithmetic."""
    q = np.float32(np.float64(pct) / 100.0)
    nf = np.float32(n)
    pos = np.float32(q * np.float32(nf - np.float32(1.0)))
    low = np.floor(pos)
    high = np.ceil(pos)
    hw = np.float32(pos - low)
    lw = np.float32(np.float32(1.0) - hw)
    low = int(min(max(low, 0.0), float(n - 1)))
    high = int(min(max(high, 0.0), float(n - 1)))
    return low, high, lw, hw


def _fallback_numpy(y_pred, y_true):
    """Exact host computation (used only if the value band misses)."""
    y_pred = y_pred.astype(np.float32)
    y_true = y_true.astype(np.float32)
    n = y_true.size
    vs = np.sort(y_true)

    def pctl(pct):
        low, high, lw, hw = _f32_percentile_pos(n, pct)
        return np.float32(
            np.float32(vs[low] * lw) + np.float32(vs[high] * hw)
        )

    lo_t = pctl(LEFT_PCT)
    hi_t = pctl(RIGHT_PCT)
    r = (y_true - y_pred).astype(np.float32)
    r2 = (r * r).astype(np.float64)
    pen = np.where((y_true < lo_t) | (y_true > hi_t), PENALTY, 1.0)
    mse = (pen * r2).mean()
    var = y_pred.astype(np.float64).var(ddof=1)
    return np.float32(mse - VAR_W * var)


def _order_stat_threshold(win_sorted, base_rank, n, pct):
    """Exact percentile from a sorted value-band slice.

    win_sorted holds (ascending) all elements with global ranks
    [base_rank, base_rank + len(win_sorted)).  Returns None if the
    percentile's order statistics are not inside the window.
    """
    low, high, lw, hw = _f32_percentile_pos(n, pct)
    i_lo = low - base_rank
    i_hi = high - base_rank
    if i_lo < 0 or i_hi < 0 or i_hi >= win_sorted.size or i_lo >= win_sorted.size:
        return None
    lv = win_sorted[i_lo]
    hv = win_sorted[i_hi]
    return np.float32(np.float32(lv * lw) + np.float32(hv * hw))


def kernel(y_pred, y_true):
    y_pred = np.asarray(y_pred, dtype=np.float32).reshape(-1)
    y_true = np.asarray(y_true, dtype=np.float32).reshape(-1)
    assert y_pred.shape == (N_TOTAL,) and y_true.shape == (N_TOTAL,)

    res = run_device(y_pred, y_true)
    s_r2, s_yp2, s_mid, c_l, c_r, s_yp = _combine(res.results)

    n = float(N_TOTAL)
    c_l = int(round(c_l))   # exact: f32 holds small integers exactly
    c_r = int(round(c_r))

    # value bands around the two percentiles (host-side ranking, o(N) output)
    band_l = np.sort(y_true[(y_true >= -T_OUT) & (y_true <= -T_IN)])
    band_r = np.sort(y_true[(y_true >= T_IN) & (y_true <= T_OUT)])

    lo_t = _order_stat_threshold(band_l, c_l, N_TOTAL, LEFT_PCT)
    base_r = N_TOTAL - c_r - band_r.size
    hi_t = _order_stat_threshold(band_r, base_r, N_TOTAL, RIGHT_PCT)

    if (
        lo_t is None
        or hi_t is None
        or not (-float(T_OUT) < lo_t < -float(T_IN))
        or not (float(T_IN) < hi_t < float(T_OUT))
    ):
        return _fallback_numpy(y_pred, y_true)

    # exact correction over the bands: device penalized |y|>T_MID, we want
    # y<lo_t or y>hi_t.  All disagreeing elements lie inside the bands.
    sel = ((y_true >= -T_OUT) & (y_true <= -T_IN)) | (
        (y_true >= T_IN) & (y_true <= T_OUT)
    )
    yb = y_true[sel]
    rb = (yb - y_pred[sel]).astype(np.float32)
    r2b = (rb * rb).astype(np.float64)
    want = (yb < lo_t) | (yb > hi_t)
    dev = np.abs(yb) > T_MID
    corr = (r2b * (want.astype(np.float64) - dev.astype(np.float64))).sum()

    tails = (s_r2 - s_mid) + corr
    mse = (s_r2 + (PENALTY - 1.0) * tails) / n
    var = (s_yp2 - (s_yp * s_yp) / n) / (n - 1.0)
    return np.float32(mse - VAR_W * var)


if __name__ == "__main__":
    rng = np.random.default_rng(0)
    yp = rng.standard_normal(N_TOTAL, dtype=np.float32)
    yt = rng.standard_normal(N_TOTAL, dtype=np.float32)
    print(kernel(yp, yt))



# revision 20
# speedup vs baseline: 1.5576x; 1.5576x over previous
"""Trainium2 Bass kernel for nn_CustomMSELoss (penalty-weighted MSE - variance).

loss = mean(penalty * (y_true - y_pred)^2) - var(y_pred, ddof=1)
  penalty = 6 where y_true < percentile(y_true, 15)
          = 6 where y_true > percentile(y_true, 85)
          = 1 otherwise

Strategy (8 NeuronCores, data-parallel over the element axis):
  Each core streams its 1/8 shard of (y_true, y_pred) once from HBM
  (both streams on the single SWDGE ring -- measured fastest; ~335 GB/s
  per core) and computes, fully fused per 128x2048 tile:
    - r = y_true - y_pred            (DVE tensor_sub)
    - |y_true|                       (ACT Abs; feeds the mid-band mask)
    - sum(r^2), sum(y_pred^2)        (squares column-SPLIT between ACT
                                      Square and DVE STT, both with fused
                                      accumulators, split tuned on HW so
                                      ACT and DVE finish together)
    - sum(r^2 * [|y_t|<=T_MID])      (DVE STT mask + accum)
    - sum(y_pred)                    (PE ones-matmul accumulating in PSUM)
  Measured fp32 rates: DVE 2-input ~105 G elem/s, DVE/ACT 1-input
  ~104-195 G elem/s, so no single engine can hide all five elementwise
  passes behind the ~100 us DMA streaming time; the column split
  balances ACT (abs + most of both squares) against DVE (sub + mask +
  the residual square columns) at ~102 us each.

  Since LEFT_PENALTY == RIGHT_PENALTY, only the combined tail sum
  sum(r^2) - sum(r^2*mid) is needed, with an exact host-side correction
  for elements near the percentile boundaries.

  The exact percentiles are order statistics.  The host counts elements
  below/above +-T_OUT (vectorized, fp32-exact compares), ranks the order
  statistic inside the narrow value band (T_IN..T_OUT, ~1% of elements)
  and applies the exact r^2 correction for elements between the fixed
  device threshold T_MID and the true percentile thresholds.  All
  arithmetic that must match the device (subtract, square, compares) is
  replayed in float32.  If the band does not contain the percentile
  ranks (pathological input distribution), falls back to an exact host
  computation.
"""

import os
import sys

import numpy as np

# ---------------------------------------------------------------- constants
N_TOTAL = 33554432
NCORES = 8
SHARD = N_TOTAL // NCORES          # 4_194_304
P = 128                            # SBUF partitions
F = 2048                           # tile free dim
NTILES = SHARD // (P * F)          # 16
MM_N = 512                         # matmul free-dim chunk

LEFT_PCT = 15.0
RIGHT_PCT = 85.0
PENALTY = 6.0
VAR_W = 1.0

# Fixed value-band thresholds around the expected +-1.0364 percentiles of
# N(0,1).  T_MID is the on-device penalty-mask boundary; the host corrects
# exactly within the (T_IN, T_OUT) band, which must contain T_MID and both
# true percentile values.
T_MID = np.float32(1.04)
T_IN = np.float32(1.025)
T_OUT = np.float32(1.055)

_CONCOURSE_PATHS = ["/opt/trn_rl_repo", "/root/.axon_site/_ro/trn_rl_repo"]


def _import_concourse():
    try:
        import concourse.bass  # noqa: F401
    except ImportError:
        for p in _CONCOURSE_PATHS:
            if os.path.isdir(p) and p not in sys.path:
                sys.path.insert(0, p)
        import concourse.bass  # noqa: F401


# ---------------------------------------------------------------- device IR
_NC_CACHE = {}

# engine assignment / buffering chosen from HW A/B timing
BEST_CFG = dict(dma_yt="gpsimd", dma_yp="gpsimd", io_bufs=4, mid_bufs=3,
                split=1920, split_p=1472)


def build_nc(shard=SHARD, f=F, ntiles=None, repeat=1,
             dma_yt="gpsimd", dma_yp="gpsimd", io_bufs=3, mid_bufs=2,
             dma_span=1, split=1536, split_p=None, dve_abs=False,
             en_dma=True, en_dve=True, en_act=True, en_pe=True):
    """Build the per-core Bass program (identical on all cores).

    repeat>1 re-runs the whole streaming pass (for HW timing via wall-clock
    deltas); outputs stay valid since accumulator slots are overwritten.
    """
    _import_concourse()
    from contextlib import ExitStack

    import concourse.bacc as bacc
    import concourse.bass as bass  # noqa: F401
    import concourse.tile as tile
    from concourse import mybir

    if ntiles is None:
        ntiles = shard // (P * f)
    assert shard == P * f * ntiles

    assert ntiles % dma_span == 0
    if split_p is None:
        split_p = split
    key = (shard, f, ntiles, repeat, dma_yt, dma_yp, io_bufs, mid_bufs,
           dma_span, split, split_p, dve_abs, en_dma, en_dve, en_act, en_pe)
    if key in _NC_CACHE:
        return _NC_CACHE[key]

    fp32 = mybir.dt.float32
    Alu = mybir.AluOpType
    Act = mybir.ActivationFunctionType

    nc = bacc.Bacc()
    yt_d = nc.declare_dram_parameter("y_true", [shard], fp32, isOutput=False)
    yp_d = nc.declare_dram_parameter("y_pred", [shard], fp32, isOutput=False)
    out_acc = nc.declare_dram_parameter("acc", [P, 5 * ntiles], fp32, isOutput=True)
    out_yps = nc.declare_dram_parameter("ypsum", [1, MM_N], fp32, isOutput=True)

    # DMA granularity: dma_span compute-tiles per dma_start
    fd = f * dma_span
    ytv = yt_d[:].rearrange("(n p f) -> n p f", p=P, f=fd)
    ypv = yp_d[:].rearrange("(n p f) -> n p f", p=P, f=fd)

    with ExitStack() as ctx:
        tc = ctx.enter_context(tile.TileContext(nc))
        io = ctx.enter_context(tc.tile_pool(name="io", bufs=io_bufs))
        mid = ctx.enter_context(tc.tile_pool(name="mid", bufs=mid_bufs))
        scr = ctx.enter_context(tc.tile_pool(name="scr", bufs=1))
        accp = ctx.enter_context(tc.tile_pool(name="accp", bufs=1))
        psp = ctx.enter_context(tc.tile_pool(name="psum", bufs=1, space="PSUM"))

        # acc layout along free dim: [r2A | r2D | yp2A | yp2D | smid] x ntiles
        # (A = columns [0:split] summed by ACT, D = [split:f] summed by DVE)
        acc = accp.tile([P, 5 * ntiles], fp32)
        if (not (en_act and en_dve) or split == 0 or split == f
                or split_p == 0 or split_p == f or dve_abs):
            nc.vector.memset(acc, 0.0)
        scr_dve = scr.tile([P, f], fp32)
        scr_act = scr.tile([P, f], fp32)
        ones = accp.tile([P, 1], fp32)
        nc.vector.memset(ones, 1.0)
        ps = psp.tile([1, MM_N], fp32)

        if repeat > 1:
            # timing builds: loop the whole streaming pass on-device so the
            # kernel's execution dominates wall-clock (constant setup above
            # stays outside the loop, matching its one-time cost in a real
            # single-shot run)
            ctx.enter_context(tc.For_i(0, repeat, 1))

        a_r2A = lambda t: acc[:, 0 * ntiles + t : 0 * ntiles + t + 1]
        a_r2D = lambda t: acc[:, 1 * ntiles + t : 1 * ntiles + t + 1]
        a_yp2A = lambda t: acc[:, 2 * ntiles + t : 2 * ntiles + t + 1]
        a_yp2D = lambda t: acc[:, 3 * ntiles + t : 3 * ntiles + t + 1]
        a_smid = lambda t: acc[:, 4 * ntiles + t : 4 * ntiles + t + 1]

        for td in range(ntiles // dma_span):
            ytd = io.tile([P, fd], fp32, tag="yt")
            ypd = io.tile([P, fd], fp32, tag="yp")
            if en_dma:
                # "alt" alternates both streams between the SWDGE and HWDGE
                # rings per DMA tile (keeps 2 MB sequential runs per ring)
                eng_t = ["gpsimd", "sync"][td % 2] if dma_yt == "alt" else dma_yt
                eng_p = ["gpsimd", "sync"][td % 2] if dma_yp == "alt" else dma_yp
                getattr(nc, eng_t).dma_start(out=ytd, in_=ytv[td])
                getattr(nc, eng_p).dma_start(out=ypd, in_=ypv[td])
            for ts in range(dma_span):
                t = td * dma_span + ts
                yt = ytd[:, ts * f : (ts + 1) * f]
                yp = ypd[:, ts * f : (ts + 1) * f]

                if dve_abs and en_dve and en_act:
                    # decoupled layout: DVE does abs (1-input tensor_scalar
                    # via abs_max) + sub + masked-mid; ACT does both full
                    # squares
                    ayt = mid.tile([P, f], fp32, tag="ayt")
                    r = mid.tile([P, f], fp32, tag="r")
                    r2 = mid.tile([P, f], fp32, tag="r2")
                    nc.vector.tensor_scalar(ayt, yt, 0.0, None,
                                            Alu.abs_max, Alu.bypass)
                    nc.vector.tensor_sub(r, yt, yp)
                    nc.scalar.activation(r2, r, Act.Square,
                                         accum_out=a_r2A(t))
                    nc.scalar.activation(scr_act, yp, Act.Square,
                                         accum_out=a_yp2A(t))
                    # smid += sum((|y_t| <= T_MID) * r2)
                    nc.vector.scalar_tensor_tensor(
                        scr_dve, ayt, float(T_MID), r2, Alu.is_le,
                        Alu.mult, accum_out=a_smid(t),
                    )
                else:
                    if en_act:
                        ayt = mid.tile([P, f], fp32, tag="ayt")
                        nc.scalar.activation(ayt, yt, Act.Abs)
                    if en_dve:
                        r = mid.tile([P, f], fp32, tag="r")
                        r2 = mid.tile([P, f], fp32, tag="r2")
                        nc.vector.tensor_sub(r, yt, yp)
                        # r2 = r*r with the reduction fused; the columns are
                        # split between ACT (Square) and DVE (STT) to balance
                        # engine time
                        if en_act and split > 0:
                            nc.scalar.activation(r2[:, 0:split], r[:, 0:split],
                                                 Act.Square, accum_out=a_r2A(t))
                        if split < f:
                            rs = r[:, split:f]
                            nc.vector.scalar_tensor_tensor(
                                r2[:, split:f], rs, 1.0, rs, Alu.mult, Alu.mult,
                                accum_out=a_r2D(t),
                            )
                        if en_act:
                            # smid += sum((|y_t| <= T_MID) * r2)
                            nc.vector.scalar_tensor_tensor(
                                scr_dve, ayt, float(T_MID), r2, Alu.is_le,
                                Alu.mult, accum_out=a_smid(t),
                            )

                    if en_act and split_p > 0:
                        nc.scalar.activation(scr_act[:, 0:split_p],
                                             yp[:, 0:split_p],
                                             Act.Square, accum_out=a_yp2A(t))
                    if en_dve and split_p < f:
                        yps_ = yp[:, split_p:f]
                        nc.vector.scalar_tensor_tensor(
                            scr_act[:, split_p:f], yps_, 1.0, yps_, Alu.mult,
                            Alu.mult, accum_out=a_yp2D(t),
                        )

                # sum(y_pred) on the otherwise-idle PE: ones^T @ yp chunks,
                # accumulated in a single PSUM region across all tiles
                n_mm = f // MM_N
                if en_pe:
                    for c in range(n_mm):
                        nc.tensor.matmul(
                            ps[:, :],
                            ones,
                            yp[:, c * MM_N : (c + 1) * MM_N],
                            start=(t == 0 and c == 0),
                            stop=(t == ntiles - 1 and c == n_mm - 1),
                        )

        # write back results (on the otherwise-idle HWDGE sync ring so the
        # SWDGE input ring is not disturbed)
        yps_sb = accp.tile([1, MM_N], fp32)
        if en_pe:
            nc.vector.tensor_copy(yps_sb, ps)
        else:
            nc.vector.memset(yps_sb, 0.0)
        nc.sync.dma_start(out=out_acc[:, :], in_=acc)
        nc.sync.dma_start(out=out_yps[:, :], in_=yps_sb)

    nc.finalize()
    _NC_CACHE[key] = nc
    return nc


# ---------------------------------------------------------------- device run
def run_device(y_pred, y_true, trace=False):
    """Shard across 8 cores, run the Bass kernel, return per-core outputs."""
    _import_concourse()
    from concourse.bass_utils import run_bass_kernel_spmd

    nc = build_nc(**BEST_CFG)
    in_maps = []
    for i in range(NCORES):
        sl = slice(i * SHARD, (i + 1) * SHARD)
        in_maps.append(
            {
                "y_true": np.ascontiguousarray(y_true[sl]),
                "y_pred": np.ascontiguousarray(y_pred[sl]),
            }
        )
    res = run_bass_kernel_spmd(nc, in_maps, list(range(NCORES)), trace=trace)
    return res


def _combine(results):
    """Combine per-core device partials (float64)."""
    acc = np.stack([np.asarray(r["acc"], dtype=np.float64) for r in results])
    nt = acc.shape[-1] // 5
    s_r2 = acc[:, :, 0 * nt : 2 * nt].sum()
    s_yp2 = acc[:, :, 2 * nt : 4 * nt].sum()
    s_mid = acc[:, :, 4 * nt : 5 * nt].sum()
    s_yp = np.stack([np.asarray(r["ypsum"], dtype=np.float64) for r in results]).sum()
    return s_r2, s_yp2, s_mid, s_yp


# ------------------------------------------------------------- host finishing
def _f32_percentile_pos(n, pct):
    """Replicate jnp.percentile's float32 position arithmetic."""
    q = np.float32(np.float64(pct) / 100.0)
    nf = np.float32(n)
    pos = np.float32(q * np.float32(nf - np.float32(1.0)))
    low = np.floor(pos)
    high = np.ceil(pos)
    hw = np.float32(pos - low)
    lw = np.float32(np.float32(1.0) - hw)
    low = int(min(max(low, 0.0), float(n - 1)))
    high = int(min(max(high, 0.0), float(n - 1)))
    return low, high, lw, hw


def _fallback_numpy(y_pred, y_true):
    """Exact host computation (used only if the value band misses)."""
    y_pred = y_pred.astype(np.float32)
    y_true = y_true.astype(np.float32)
    n = y_true.size
    vs = np.sort(y_true)

    def pctl(pct):
        low, high, lw, hw = _f32_percentile_pos(n, pct)
        return np.float32(
            np.float32(vs[low] * lw) + np.float32(vs[high] * hw)
        )

    lo_t = pctl(LEFT_PCT)
    hi_t = pctl(RIGHT_PCT)
    r = (y_true - y_pred).astype(np.float32)
    r2 = (r * r).astype(np.float64)
    pen = np.where((y_true < lo_t) | (y_true > hi_t), PENALTY, 1.0)
    mse = (pen * r2).mean()
    var = y_pred.astype(np.float64).var(ddof=1)
    return np.float32(mse - VAR_W * var)


def _order_stat_threshold(win_sorted, base_rank, n, pct):
    """Exact percentile from a sorted value-band slice.

    win_sorted holds (ascending) all elements with global ranks
    [base_rank, base_rank + len(win_sorted)).  Returns None if the
    percentile's order statistics are not inside the window.
    """
    low, high, lw, hw = _f32_percentile_pos(n, pct)
    i_lo = low - base_rank
    i_hi = high - base_rank
    if i_lo < 0 or i_hi < 0 or i_hi >= win_sorted.size or i_lo >= win_sorted.size:
        return None
    lv = win_sorted[i_lo]
    hv = win_sorted[i_hi]
    return np.float32(np.float32(lv * lw) + np.float32(hv * hw))


def kernel(y_pred, y_true):
    y_pred = np.asarray(y_pred, dtype=np.float32).reshape(-1)
    y_true = np.asarray(y_true, dtype=np.float32).reshape(-1)
    assert y_pred.shape == (N_TOTAL,) and y_true.shape == (N_TOTAL,)

    res = run_device(y_pred, y_true)
    s_r2, s_yp2, s_mid, s_yp = _combine(res.results)

    n = float(N_TOTAL)
    # exact tail counts (vectorized host pass; fp32 compares match device)
    c_l = int(np.count_nonzero(y_true < -T_OUT))
    c_r = int(np.count_nonzero(y_true > T_OUT))

    # value bands around the two percentiles (host-side ranking, o(N) output)
    band_l = np.sort(y_true[(y_true >= -T_OUT) & (y_true <= -T_IN)])
    band_r = np.sort(y_true[(y_true >= T_IN) & (y_true <= T_OUT)])

    lo_t = _order_stat_threshold(band_l, c_l, N_TOTAL, LEFT_PCT)
    base_r = N_TOTAL - c_r - band_r.size
    hi_t = _order_stat_threshold(band_r, base_r, N_TOTAL, RIGHT_PCT)

    if (
        lo_t is None
        or hi_t is None
        or not (-float(T_OUT) < lo_t < -float(T_IN))
        or not (float(T_IN) < hi_t < float(T_OUT))
    ):
        return _fallback_numpy(y_pred, y_true)

    # exact correction over the bands: device penalized y outside
    # [-T_MID, T_MID]; we want y<lo_t or y>hi_t.  All disagreeing elements
    # lie inside the bands.
    sel = ((y_true >= -T_OUT) & (y_true <= -T_IN)) | (
        (y_true >= T_IN) & (y_true <= T_OUT)
    )
    yb = y_true[sel]
    rb = (yb - y_pred[sel]).astype(np.float32)
    r2b = (rb * rb).astype(np.float64)
    want = (yb < lo_t) | (yb > hi_t)
    dev = np.abs(yb) > T_MID
    corr = (r2b * (want.astype(np.float64) - dev.astype(np.float64))).sum()

    tails = (s_r2 - s_mid) + corr
    mse = (s_r2 + (PENALTY - 1.0) * tails) / n
    var = (s_yp2 - (s_yp * s_yp) / n) / (n - 1.0)
    return np.float32(mse - VAR_W * var)


if __name__ == "__main__":
    rng = np.random.default_rng(0)
    yp = rng.standard_normal(N_TOTAL, dtype=np.float32)
    yt = rng.standard_normal(N_TOTAL, dtype=np.float32)
    print(kernel(yp, yt))


# revision 21
# speedup vs baseline: 1.5982x; 1.0260x over previous
"""Trainium2 Bass kernel for nn_CustomMSELoss (penalty-weighted MSE - variance).

loss = mean(penalty * (y_true - y_pred)^2) - var(y_pred, ddof=1)
  penalty = 6 where y_true < percentile(y_true, 15)
          = 6 where y_true > percentile(y_true, 85)
          = 1 otherwise

Strategy (8 NeuronCores, data-parallel over the element axis):
  Each core streams its 1/8 shard of (y_true, y_pred) once from HBM
  (both streams on the single SWDGE ring -- measured fastest; ~335 GB/s
  per core) and computes, fully fused per 128x2048 tile:
    - r = y_true - y_pred            (DVE tensor_sub)
    - |y_true|                       (ACT Abs; feeds the mid-band mask)
    - sum(r^2), sum(y_pred^2)        (squares column-SPLIT between ACT
                                      Square and DVE STT, both with fused
                                      accumulators, split tuned on HW so
                                      ACT and DVE finish together)
    - sum(r^2 * [|y_t|<=T_MID])      (DVE STT mask + accum)
    - sum(y_pred)                    (PE ones-matmul accumulating in PSUM)
  Measured fp32 rates: DVE 2-input ~105 G elem/s, DVE/ACT 1-input
  ~104-195 G elem/s, so no single engine can hide all five elementwise
  passes behind the ~100 us DMA streaming time; the column split
  balances ACT (abs + most of both squares) against DVE (sub + mask +
  the residual square columns) at ~102 us each.

  Since LEFT_PENALTY == RIGHT_PENALTY, only the combined tail sum
  sum(r^2) - sum(r^2*mid) is needed, with an exact host-side correction
  for elements near the percentile boundaries.

  The exact percentiles are order statistics.  The host counts elements
  below/above +-T_OUT (vectorized, fp32-exact compares), ranks the order
  statistic inside the narrow value band (T_IN..T_OUT, ~1% of elements)
  and applies the exact r^2 correction for elements between the fixed
  device threshold T_MID and the true percentile thresholds.  All
  arithmetic that must match the device (subtract, square, compares) is
  replayed in float32.  If the band does not contain the percentile
  ranks (pathological input distribution), falls back to an exact host
  computation.
"""

import os
import sys

import numpy as np

# ---------------------------------------------------------------- constants
N_TOTAL = 33554432
NCORES = 8
SHARD = N_TOTAL // NCORES          # 4_194_304
P = 128                            # SBUF partitions
F = 2048                           # tile free dim
NTILES = SHARD // (P * F)          # 16
MM_N = 512                         # matmul free-dim chunk

LEFT_PCT = 15.0
RIGHT_PCT = 85.0
PENALTY = 6.0
VAR_W = 1.0

# Fixed value-band thresholds around the expected +-1.0364 percentiles of
# N(0,1).  T_MID is the on-device penalty-mask boundary; the host corrects
# exactly within the (T_IN, T_OUT) band, which must contain T_MID and both
# true percentile values.
T_MID = np.float32(1.04)
T_IN = np.float32(1.025)
T_OUT = np.float32(1.055)

_CONCOURSE_PATHS = ["/opt/trn_rl_repo", "/root/.axon_site/_ro/trn_rl_repo"]


def _import_concourse():
    try:
        import concourse.bass  # noqa: F401
    except ImportError:
        for p in _CONCOURSE_PATHS:
            if os.path.isdir(p) and p not in sys.path:
                sys.path.insert(0, p)
        import concourse.bass  # noqa: F401


# ---------------------------------------------------------------- device IR
_NC_CACHE = {}

# engine assignment / buffering chosen from HW A/B timing
BEST_CFG = dict(dma_yt="gpsimd", dma_yp="gpsimd", io_bufs=4, mid_bufs=3,
                split=1920, split_p=1472)


def build_nc(shard=SHARD, f=F, ntiles=None, repeat=1,
             dma_yt="gpsimd", dma_yp="gpsimd", io_bufs=3, mid_bufs=2,
             dma_span=1, split=1536, split_p=None, dve_abs=False,
             en_dma=True, en_dve=True, en_act=True, en_pe=True):
    """Build the per-core Bass program (identical on all cores).

    repeat>1 re-runs the whole streaming pass (for HW timing via wall-clock
    deltas); outputs stay valid since accumulator slots are overwritten.
    """
    _import_concourse()
    from contextlib import ExitStack

    import concourse.bacc as bacc
    import concourse.bass as bass  # noqa: F401
    import concourse.tile as tile
    from concourse import mybir

    if ntiles is None:
        ntiles = shard // (P * f)
    assert shard == P * f * ntiles

    assert ntiles % dma_span == 0
    if split_p is None:
        split_p = split
    key = (shard, f, ntiles, repeat, dma_yt, dma_yp, io_bufs, mid_bufs,
           dma_span, split, split_p, dve_abs, en_dma, en_dve, en_act, en_pe)
    if key in _NC_CACHE:
        return _NC_CACHE[key]

    fp32 = mybir.dt.float32
    Alu = mybir.AluOpType
    Act = mybir.ActivationFunctionType

    nc = bacc.Bacc()
    yt_d = nc.declare_dram_parameter("y_true", [shard], fp32, isOutput=False)
    yp_d = nc.declare_dram_parameter("y_pred", [shard], fp32, isOutput=False)
    out_acc = nc.declare_dram_parameter("acc", [P, 5 * ntiles], fp32, isOutput=True)
    out_yps = nc.declare_dram_parameter("ypsum", [1, MM_N], fp32, isOutput=True)

    # DMA granularity: dma_span compute-tiles per dma_start
    fd = f * dma_span
    ytv = yt_d[:].rearrange("(n p f) -> n p f", p=P, f=fd)
    ypv = yp_d[:].rearrange("(n p f) -> n p f", p=P, f=fd)

    with ExitStack() as ctx:
        tc = ctx.enter_context(tile.TileContext(nc))
        io = ctx.enter_context(tc.tile_pool(name="io", bufs=io_bufs))
        mid = ctx.enter_context(tc.tile_pool(name="mid", bufs=mid_bufs))
        scr = ctx.enter_context(tc.tile_pool(name="scr", bufs=1))
        accp = ctx.enter_context(tc.tile_pool(name="accp", bufs=1))
        psp = ctx.enter_context(tc.tile_pool(name="psum", bufs=1, space="PSUM"))

        # acc layout along free dim: [r2A | r2D | yp2A | yp2D | smid] x ntiles
        # (A = columns [0:split] summed by ACT, D = [split:f] summed by DVE)
        acc = accp.tile([P, 5 * ntiles], fp32)
        if (not (en_act and en_dve) or split == 0 or split == f
                or split_p == 0 or split_p == f or dve_abs):
            nc.vector.memset(acc, 0.0)
        scr_dve = scr.tile([P, f], fp32)
        scr_act = scr.tile([P, f], fp32)
        ones = accp.tile([P, 1], fp32)
        nc.vector.memset(ones, 1.0)
        ps = psp.tile([1, MM_N], fp32)

        from contextlib import ExitStack as _ES
        lctx = ctx.enter_context(_ES())
        if repeat > 1:
            # timing builds: loop the whole streaming pass on-device so the
            # kernel's execution dominates wall-clock (constant setup above
            # and the result writeback below stay outside the loop, matching
            # their one-time cost in a real single-shot run; accumulator
            # slots are overwritten every iteration so the final values are
            # intact when the epilogue reads them)
            lctx.enter_context(tc.For_i(0, repeat, 1))

        a_r2A = lambda t: acc[:, 0 * ntiles + t : 0 * ntiles + t + 1]
        a_r2D = lambda t: acc[:, 1 * ntiles + t : 1 * ntiles + t + 1]
        a_yp2A = lambda t: acc[:, 2 * ntiles + t : 2 * ntiles + t + 1]
        a_yp2D = lambda t: acc[:, 3 * ntiles + t : 3 * ntiles + t + 1]
        a_smid = lambda t: acc[:, 4 * ntiles + t : 4 * ntiles + t + 1]

        for td in range(ntiles // dma_span):
            ytd = io.tile([P, fd], fp32, tag="yt")
            ypd = io.tile([P, fd], fp32, tag="yp")
            if en_dma:
                # "alt" alternates both streams between the SWDGE and HWDGE
                # rings per DMA tile (keeps 2 MB sequential runs per ring)
                eng_t = ["gpsimd", "sync"][td % 2] if dma_yt == "alt" else dma_yt
                eng_p = ["gpsimd", "sync"][td % 2] if dma_yp == "alt" else dma_yp
                getattr(nc, eng_t).dma_start(out=ytd, in_=ytv[td])
                getattr(nc, eng_p).dma_start(out=ypd, in_=ypv[td])
            for ts in range(dma_span):
                t = td * dma_span + ts
                yt = ytd[:, ts * f : (ts + 1) * f]
                yp = ypd[:, ts * f : (ts + 1) * f]

                if dve_abs and en_dve and en_act:
                    # decoupled layout: DVE does abs (1-input tensor_scalar
                    # via abs_max) + sub + masked-mid; ACT does both full
                    # squares
                    ayt = mid.tile([P, f], fp32, tag="ayt")
                    r = mid.tile([P, f], fp32, tag="r")
                    r2 = mid.tile([P, f], fp32, tag="r2")
                    nc.vector.tensor_scalar(ayt, yt, 0.0, None,
                                            Alu.abs_max, Alu.bypass)
                    nc.vector.tensor_sub(r, yt, yp)
                    nc.scalar.activation(r2, r, Act.Square,
                                         accum_out=a_r2A(t))
                    nc.scalar.activation(scr_act, yp, Act.Square,
                                         accum_out=a_yp2A(t))
                    # smid += sum((|y_t| <= T_MID) * r2)
                    nc.vector.scalar_tensor_tensor(
                        scr_dve, ayt, float(T_MID), r2, Alu.is_le,
                        Alu.mult, accum_out=a_smid(t),
                    )
                else:
                    if en_act:
                        ayt = mid.tile([P, f], fp32, tag="ayt")
                        nc.scalar.activation(ayt, yt, Act.Abs)
                    if en_dve:
                        r = mid.tile([P, f], fp32, tag="r")
                        r2 = mid.tile([P, f], fp32, tag="r2")
                        nc.vector.tensor_sub(r, yt, yp)
                        # r2 = r*r with the reduction fused; the columns are
                        # split between ACT (Square) and DVE (STT) to balance
                        # engine time
                        if en_act and split > 0:
                            nc.scalar.activation(r2[:, 0:split], r[:, 0:split],
                                                 Act.Square, accum_out=a_r2A(t))
                        if split < f:
                            rs = r[:, split:f]
                            nc.vector.scalar_tensor_tensor(
                                r2[:, split:f], rs, 1.0, rs, Alu.mult, Alu.mult,
                                accum_out=a_r2D(t),
                            )
                        if en_act:
                            # smid += sum((|y_t| <= T_MID) * r2)
                            nc.vector.scalar_tensor_tensor(
                                scr_dve, ayt, float(T_MID), r2, Alu.is_le,
                                Alu.mult, accum_out=a_smid(t),
                            )

                    if en_act and split_p > 0:
                        nc.scalar.activation(scr_act[:, 0:split_p],
                                             yp[:, 0:split_p],
                                             Act.Square, accum_out=a_yp2A(t))
                    if en_dve and split_p < f:
                        yps_ = yp[:, split_p:f]
                        nc.vector.scalar_tensor_tensor(
                            scr_act[:, split_p:f], yps_, 1.0, yps_, Alu.mult,
                            Alu.mult, accum_out=a_yp2D(t),
                        )

                # sum(y_pred) on the otherwise-idle PE: ones^T @ yp chunks,
                # accumulated in a single PSUM region across all tiles
                n_mm = f // MM_N
                if en_pe:
                    for c in range(n_mm):
                        nc.tensor.matmul(
                            ps[:, :],
                            ones,
                            yp[:, c * MM_N : (c + 1) * MM_N],
                            start=(t == 0 and c == 0),
                            stop=(t == ntiles - 1 and c == n_mm - 1),
                        )

        lctx.close()
        # write back results (on the otherwise-idle HWDGE sync ring so the
        # SWDGE input ring is not disturbed)
        yps_sb = accp.tile([1, MM_N], fp32)
        if en_pe:
            nc.vector.tensor_copy(yps_sb, ps)
        else:
            nc.vector.memset(yps_sb, 0.0)
        nc.sync.dma_start(out=out_acc[:, :], in_=acc)
        nc.sync.dma_start(out=out_yps[:, :], in_=yps_sb)

    nc.finalize()
    _NC_CACHE[key] = nc
    return nc


# ---------------------------------------------------------------- device run
def run_device(y_pred, y_true, trace=False):
    """Shard across 8 cores, run the Bass kernel, return per-core outputs."""
    _import_concourse()
    from concourse.bass_utils import run_bass_kernel_spmd

    nc = build_nc(**BEST_CFG)
    in_maps = []
    for i in range(NCORES):
        sl = slice(i * SHARD, (i + 1) * SHARD)
        in_maps.append(
            {
                "y_true": np.ascontiguousarray(y_true[sl]),
                "y_pred": np.ascontiguousarray(y_pred[sl]),
            }
        )
    res = run_bass_kernel_spmd(nc, in_maps, list(range(NCORES)), trace=trace)
    return res


def _combine(results):
    """Combine per-core device partials (float64)."""
    acc = np.stack([np.asarray(r["acc"], dtype=np.float64) for r in results])
    nt = acc.shape[-1] // 5
    s_r2 = acc[:, :, 0 * nt : 2 * nt].sum()
    s_yp2 = acc[:, :, 2 * nt : 4 * nt].sum()
    s_mid = acc[:, :, 4 * nt : 5 * nt].sum()
    s_yp = np.stack([np.asarray(r["ypsum"], dtype=np.float64) for r in results]).sum()
    return s_r2, s_yp2, s_mid, s_yp


# ------------------------------------------------------------- host finishing
def _f32_percentile_pos(n, pct):
    """Replicate jnp.percentile's float32 position arithmetic."""
    q = np.float32(np.float64(pct) / 100.0)
    nf = np.float32(n)
    pos = np.float32(q * np.float32(nf - np.float32(1.0)))
    low = np.floor(pos)
    high = np.ceil(pos)
    hw = np.float32(pos - low)
    lw = np.float32(np.float32(1.0) - hw)
    low = int(min(max(low, 0.0), float(n - 1)))
    high = int(min(max(high, 0.0), float(n - 1)))
    return low, high, lw, hw


def _fallback_numpy(y_pred, y_true):
    """Exact host computation (used only if the value band misses)."""
    y_pred = y_pred.astype(np.float32)
    y_true = y_true.astype(np.float32)
    n = y_true.size
    vs = np.sort(y_true)

    def pctl(pct):
        low, high, lw, hw = _f32_percentile_pos(n, pct)
        return np.float32(
            np.float32(vs[low] * lw) + np.float32(vs[high] * hw)
        )

    lo_t = pctl(LEFT_PCT)
    hi_t = pctl(RIGHT_PCT)
    r = (y_true - y_pred).astype(np.float32)
    r2 = (r * r).astype(np.float64)
    pen = np.where((y_true < lo_t) | (y_true > hi_t), PENALTY, 1.0)
    mse = (pen * r2).mean()
    var = y_pred.astype(np.float64).var(ddof=1)
    return np.float32(mse - VAR_W * var)


def _order_stat_threshold(win_sorted, base_rank, n, pct):
    """Exact percentile from a sorted value-band slice.

    win_sorted holds (ascending) all elements with global ranks
    [base_rank, base_rank + len(win_sorted)).  Returns None if the
    percentile's order statistics are not inside the window.
    """
    low, high, lw, hw = _f32_percentile_pos(n, pct)
    i_lo = low - base_rank
    i_hi = high - base_rank
    if i_lo < 0 or i_hi < 0 or i_hi >= win_sorted.size or i_lo >= win_sorted.size:
        return None
    lv = win_sorted[i_lo]
    hv = win_sorted[i_hi]
    return np.float32(np.float32(lv * lw) + np.float32(hv * hw))


def kernel(y_pred, y_true):
    y_pred = np.asarray(y_pred, dtype=np.float32).reshape(-1)
    y_true = np.asarray(y_true, dtype=np.float32).reshape(-1)
    assert y_pred.shape == (N_TOTAL,) and y_true.shape == (N_TOTAL,)

    res = run_device(y_pred, y_true)
    s_r2, s_yp2, s_mid, s_yp = _combine(res.results)

    n = float(N_TOTAL)
    # exact tail counts (vectorized host pass; fp32 compares match device)
    c_l = int(np.count_nonzero(y_true < -T_OUT))
    c_r = int(np.count_nonzero(y_true > T_OUT))

    # value bands around the two percentiles (host-side ranking, o(N) output)
    band_l = np.sort(y_true[(y_true >= -T_OUT) & (y_true <= -T_IN)])
    band_r = np.sort(y_true[(y_true >= T_IN) & (y_true <= T_OUT)])

    lo_t = _order_stat_threshold(band_l, c_l, N_TOTAL, LEFT_PCT)
    base_r = N_TOTAL - c_r - band_r.size
    hi_t = _order_stat_threshold(band_r, base_r, N_TOTAL, RIGHT_PCT)

    if (
        lo_t is None
        or hi_t is None
        or not (-float(T_OUT) < lo_t < -float(T_IN))
        or not (float(T_IN) < hi_t < float(T_OUT))
    ):
        return _fallback_numpy(y_pred, y_true)

    # exact correction over the bands: device penalized y outside
    # [-T_MID, T_MID]; we want y<lo_t or y>hi_t.  All disagreeing elements
    # lie inside the bands.
    sel = ((y_true >= -T_OUT) & (y_true <= -T_IN)) | (
        (y_true >= T_IN) & (y_true <= T_OUT)
    )
    yb = y_true[sel]
    rb = (yb - y_pred[sel]).astype(np.float32)
    r2b = (rb * rb).astype(np.float64)
    want = (yb < lo_t) | (yb > hi_t)
    dev = np.abs(yb) > T_MID
    corr = (r2b * (want.astype(np.float64) - dev.astype(np.float64))).sum()

    tails = (s_r2 - s_mid) + corr
    mse = (s_r2 + (PENALTY - 1.0) * tails) / n
    var = (s_yp2 - (s_yp * s_yp) / n) / (n - 1.0)
    return np.float32(mse - VAR_W * var)


if __name__ == "__main__":
    rng = np.random.default_rng(0)
    yp = rng.standard_normal(N_TOTAL, dtype=np.float32)
    yt = rng.standard_normal(N_TOTAL, dtype=np.float32)
    print(kernel(yp, yt))
